# revision 6
# baseline (speedup 1.0000x reference)
"""XNOR/ReActNet binarized 3x3 conv on 8 Trainium2 NeuronCores.

out = conv2d(sign(x - alpha), sign(weight), stride 1, pad 1)
  x      [32, 256, 56, 56] f32
  alpha  [256, 1, 1]       f32
  weight [256, 256, 3, 3]  f32
  out    [32, 256, 56, 56] f32

Strategy (data-parallel): each core takes 4 images. Binarized values are
exactly +-1, so activations/weights are cast to bf16 (exact) and the conv
runs as 9 shifted matmuls (one per kernel tap) accumulating over C_in in
fp32 PSUM. All sums are small integers -> bit-exact vs the f32 reference.

Per core:
  - weights: DMA [128co, 2304] f32, Sign->bf16, PE-transpose 36 [ci,co]
    128x128 tiles (one per (kh,kw,ci_chunk,co_chunk)).
  - activations: DMA [128ci, 3136] f32 per (img, ci_chunk), Sign(x-alpha)
    written into the interior of a zeroed [128, 58*58] padded bf16 image.
  - conv: for (img, co_chunk): 7 PSUM tiles [128, 8*56]; accumulate
    18 matmuls each (9 taps x 2 ci chunks), rhs = shifted window AP into
    the padded image; copy PSUM->SBUF staging, one 1.6MB DMA per (img,co).
"""

import numpy as np

import concourse.bass as bass
import concourse.mybir as mybir
import concourse.tile as tile
from concourse.masks import make_identity
from concourse.vector_clock import ScopedClock
from concourse.bass_utils import run_bass_kernel_spmd

N_CORES = 8
B, C, H, W, KS = 32, 256, 56, 56, 3
BL = B // N_CORES           # images per core
HP, WP = H + 2, W + 2       # padded 58x58
NPIX = H * W                # 3136
RPC = 8                     # output rows per PSUM tile
NSP = H // RPC              # 7 spatial chunks
NFREE = RPC * W             # 448 (fits one 2KB f32 PSUM bank)
NCH = C // 128              # 2 channel chunks
F32 = mybir.dt.float32
BF16 = mybir.dt.bfloat16
SIGN = mybir.ActivationFunctionType.Sign


def _split_excess_waits(nc):
    """This walrus build rejects instructions carrying more sem waits than
    the ISA sync struct holds ("Too many sync wait commands"): 2 for
    regular engine ops, 1 for control ops (Drain/NoOp). Tile's scheduler
    can attach more. Hoist the excess onto same-engine NoOps placed just
    before the instruction — engines are in-order, so semantics match."""
    k = 0
    for f in nc.m.functions:
        for bb in f.blocks:
            old = list(bb.instructions)
            new = []
            changed = False
            for ins in old:
                si = ins.sync_info
                waits = list(si.on_wait) if si and si.on_wait else []
                limit = 1
                if len(waits) > limit:
                    for w in waits[:-limit]:
                        nop = mybir.InstNoOp(
                            name=f"I-wsplit{k}",
                            ins=[],
                            outs=[],
                            engine=ins.engine,
                            sync_info=mybir.SyncInfo(on_wait=[w], on_update=[]),
                        )
                        k += 1
                        new.append(nop)
                    si.on_wait = waits[-limit:]
                    changed = True
                new.append(ins)
            if changed:
                bb.instructions[:] = new


def _build_program() -> bass.Bass:
    nc = bass.Bass()
    x = nc.dram_tensor("x", [BL, C, H, W], F32, kind="ExternalInput")
    alpha = nc.dram_tensor("alpha", [C], F32, kind="ExternalInput")
    weight = nc.dram_tensor("weight", [C, C, KS, KS], F32, kind="ExternalInput")
    out = nc.dram_tensor("out", [BL, C, H, W], F32, kind="ExternalOutput")

    xv = x[:].rearrange("b c h w -> b c (h w)")
    wv = weight[:].rearrange("o i kh kw -> o (i kh kw)")
    ov = out[:].rearrange("b c h w -> b c (h w)")

    with tile.TileContext(nc) as tc:
        with (
            tc.tile_pool(name="const", bufs=1) as constp,
            tc.tile_pool(name="apad", bufs=1) as apadp,
            tc.tile_pool(name="wsb", bufs=1) as wsbp,
            tc.tile_pool(name="xs", bufs=3) as xsp,
            tc.tile_pool(name="outs", bufs=2) as outsp,
        ):
            ident = constp.tile([128, 128], BF16, tag="ident")
            make_identity(nc, ident[:])

            alpha_sb = constp.tile([128, NCH], F32, tag="alpha")
            nc.sync.dma_start(alpha_sb[:], alpha[:].rearrange("(n p) -> p n", p=128))
            neg_alpha = constp.tile([128, NCH], F32, tag="nalpha")
            nc.scalar.mul(neg_alpha[:], alpha_sb[:], -1.0)

            # Binarized, transposed weight tiles: wT[(kh,kw,ci,co)] = [ci,co] bf16
            wT = {}
            with tc.tile_pool(name="pswt", bufs=2, space="PSUM") as pswt:
                for co in range(NCH):
                    wraw = wsbp.tile([128, C * KS * KS], F32, tag=f"wraw{co}")
                    nc.sync.dma_start(wraw[:], wv[co * 128:(co + 1) * 128, :])
                    wbin = wsbp.tile([128, C * KS * KS], BF16, tag=f"wbin{co}")
                    nc.scalar.activation(wbin[:], wraw[:], SIGN)
                    wb3 = wbin[:].rearrange("p (c k) -> p c k", c=C)
                    for ci in range(NCH):
                        for kh in range(KS):
                            for kw in range(KS):
                                pt = pswt.tile([128, 128], BF16, tag="tp")
                                src = wb3[:, ci * 128:(ci + 1) * 128, kh * KS + kw]
                                nc.tensor.transpose(pt[:], src, ident[:])
                                dst = constp.tile(
                                    [128, 128], BF16, tag=f"wT{co}_{ci}_{kh}_{kw}"
                                )
                                nc.vector.tensor_copy(dst[:], pt[:])
                                wT[(kh, kw, ci, co)] = dst

            # Padded binarized activations, bf16 [128, 58*58] per (img, ci_chunk)
            apad = {}
            for img in range(BL):
                for ci in range(NCH):
                    ap_t = apadp.tile([128, HP * WP], BF16, tag=f"ap{img}_{ci}")
                    nc.gpsimd.memset(ap_t[:], 0.0)
                    xs_t = xsp.tile([128, NPIX], F32, tag="xs")
                    nc.sync.dma_start(xs_t[:], xv[img, ci * 128:(ci + 1) * 128, :])
                    ap3 = ap_t[:].rearrange("p (h w) -> p h w", h=HP)
                    nc.scalar.activation(
                        ap3[:, 1:H + 1, 1:W + 1],
                        xs_t[:].rearrange("p (h w) -> p h w", h=H),
                        SIGN,
                        bias=neg_alpha[:, ci:ci + 1],
                    )
                    apad[(img, ci)] = ap_t

            # Conv: 9 shifted matmuls x 2 ci chunks accumulated in PSUM
            n_acc = KS * KS * NCH
            with tc.tile_pool(name="psc", bufs=8, space="PSUM") as psc:
                for img in range(BL):
                    for co in range(NCH):
                        pts = [
                            psc.tile([128, NFREE], F32, tag="conv", name=f"conv{sp}")
                            for sp in range(NSP)
                        ]
                        i_acc = 0
                        for kh in range(KS):
                            for kw in range(KS):
                                for ci in range(NCH):
                                    lhsT = wT[(kh, kw, ci, co)]
                                    a3 = apad[(img, ci)][:].rearrange(
                                        "p (h w) -> p h w", h=HP
                                    )
                                    first = i_acc == 0
                                    last = i_acc == n_acc - 1
                                    for sp in range(NSP):
                                        r0 = sp * RPC + kh
                                        rhs = a3[:, r0:r0 + RPC, kw:kw + W]
                                        nc.tensor.matmul(
                                            pts[sp][:], lhsT[:], rhs,
                                            start=first, stop=last,
                                        )
                                    i_acc += 1
                        ot = outsp.tile([128, NPIX], F32, tag="out")
                        for sp in range(NSP):
                            nc.vector.tensor_copy(
                                ot[:, sp * NFREE:(sp + 1) * NFREE], pts[sp][:]
                            )
                        nc.sync.dma_start(
                            ov[img, co * 128:(co + 1) * 128, :], ot[:]
                        )
    _split_excess_waits(nc)
    return nc


_prog_cache = {}


def _get_program() -> bass.Bass:
    if "nc" not in _prog_cache:
        _prog_cache["nc"] = _build_program()
    return _prog_cache["nc"]


def _run(x, alpha, weight, trace=False):
    x = np.ascontiguousarray(np.asarray(x, dtype=np.float32))
    alpha = np.ascontiguousarray(np.asarray(alpha, dtype=np.float32).reshape(C))
    weight = np.ascontiguousarray(np.asarray(weight, dtype=np.float32))
    assert x.shape == (B, C, H, W) and weight.shape == (C, C, KS, KS)

    nc = _get_program()
    in_maps = [
        {
            "x": np.ascontiguousarray(x[i * BL:(i + 1) * BL]),
            "alpha": alpha,
            "weight": weight,
        }
        for i in range(N_CORES)
    ]
    res = run_bass_kernel_spmd(nc, in_maps, list(range(N_CORES)), trace=trace)
    out = np.concatenate([res.results[i]["out"] for i in range(N_CORES)], axis=0)
    return out.astype(np.float32, copy=False), res


def kernel(x, alpha, weight):
    out, _ = _run(x, alpha, weight, trace=False)
    return out


def kernel_timed(x, alpha, weight):
    out, res = _run(x, alpha, weight, trace=True)
    return out, res


# revision 8
# speedup vs baseline: 1.6439x; 1.6439x over previous
"""XNOR/ReActNet binarized 3x3 conv on 8 Trainium2 NeuronCores.

out = conv2d(sign(x - alpha), sign(weight), stride 1, pad 1)
  x      [32, 256, 56, 56] f32
  alpha  [256, 1, 1]       f32
  weight [256, 256, 3, 3]  f32
  out    [32, 256, 56, 56] f32

Strategy (data-parallel): each core takes 4 images. Binarized values are
exactly +-1, so they are exact in fp8e4; the conv runs as 9 shifted
matmuls (one per kernel tap) in fp8 DoubleRow mode (contraction over all
256 input channels per matmul: 128 partitions x 2 k-tiles), accumulating
in fp32 PSUM. All sums are small integers -> bit-exact vs the reference.

Per core:
  - weights: DMA [128co, 2304] f32, Sign->bf16, PE-transpose to [ci,co]
    128x128 tiles, pack as fp8 [128, 2(ci chunk), 128co] per (kh,kw,co).
  - activations: DMA [128ci, 3136] f32 per (img, ci_chunk); Sign(x-alpha)
    written fp8 into the interior of a zeroed row-padded image
    [58 rows x 64 cols] (row stride 64 keeps the DoubleRow k-tile step
    16B-aligned); both ci chunks live in one tile -> 4D windowed rhs AP.
  - conv: per (img, co_chunk): 7 PSUM tiles [128, 8*56]; 9 accumulating
    DoubleRow matmuls each; DVE copy PSUM->SBUF; one 1.6MB DMA out.
"""

import numpy as np

import concourse.bass as bass
import concourse.mybir as mybir
import concourse.tile as tile
from concourse.masks import make_identity
from concourse.bass_utils import run_bass_kernel_spmd

N_CORES = 8
B, C, H, W, KS = 32, 256, 56, 56, 3
BL = B // N_CORES           # images per core
PH, PW = H + 2, 64          # padded rows, row stride (58 x 64)
PADF = PH * PW              # 3712 (16B aligned for DoubleRow k-tile step)
NPIX = H * W                # 3136
RPC = 8                     # output rows per PSUM tile
NSP = H // RPC              # 7 spatial chunks
NFREE = RPC * W             # 448 (fits one 2KB f32 PSUM bank)
NCH = C // 128              # 2 channel chunks
F32 = mybir.dt.float32
BF16 = mybir.dt.bfloat16
FP8 = mybir.dt.float8e4
SIGN = mybir.ActivationFunctionType.Sign
DR = mybir.MatmulPerfMode.DoubleRow


def _split_excess_waits(nc):
    """This walrus build rejects instructions carrying more than one sem
    wait ("Too many sync wait commands" from setupSyncWait). Tile's
    scheduler can attach several. Hoist the excess onto same-engine NoOps
    placed just before the instruction — engines are in-order, so the
    semantics are identical."""
    k = 0
    for f in nc.m.functions:
        for bb in f.blocks:
            old = list(bb.instructions)
            new = []
            changed = False
            for ins in old:
                si = ins.sync_info
                waits = list(si.on_wait) if si and si.on_wait else []
                if len(waits) > 1:
                    for w in waits[:-1]:
                        nop = mybir.InstNoOp(
                            name=f"I-wsplit{k}",
                            ins=[],
                            outs=[],
                            engine=ins.engine,
                            sync_info=mybir.SyncInfo(on_wait=[w], on_update=[]),
                        )
                        k += 1
                        new.append(nop)
                    si.on_wait = waits[-1:]
                    changed = True
                new.append(ins)
            if changed:
                bb.instructions[:] = new


def _build_program() -> bass.Bass:
    nc = bass.Bass()
    x = nc.dram_tensor("x", [BL, C, H, W], F32, kind="ExternalInput")
    alpha = nc.dram_tensor("alpha", [C], F32, kind="ExternalInput")
    weight = nc.dram_tensor("weight", [C, C, KS, KS], F32, kind="ExternalInput")
    out = nc.dram_tensor("out", [BL, C, H, W], F32, kind="ExternalOutput")

    xv = x[:].rearrange("b c h w -> b c (h w)")
    wv = weight[:].rearrange("o i kh kw -> o (i kh kw)")
    ov = out[:].rearrange("b c h w -> b c (h w)")

    with tile.TileContext(nc) as tc:
        with (
            tc.tile_pool(name="const", bufs=1) as constp,
            tc.tile_pool(name="apad", bufs=1) as apadp,
            tc.tile_pool(name="wsb", bufs=1) as wsbp,
            tc.tile_pool(name="xs", bufs=3) as xsp,
            tc.tile_pool(name="outs", bufs=2) as outsp,
        ):
            ident = constp.tile([128, 128], BF16, tag="ident")
            make_identity(nc, ident[:])

            alpha_sb = constp.tile([128, NCH], F32, tag="alpha")
            nc.sync.dma_start(alpha_sb[:], alpha[:].rearrange("(n p) -> p n", p=128))
            neg_alpha = constp.tile([128, NCH], F32, tag="nalpha")
            nc.scalar.mul(neg_alpha[:], alpha_sb[:], -1.0)

            # Binarized transposed weights packed for DoubleRow:
            # wdr[(kh,kw,co)] = fp8 [128 ci_local, 2 ci_chunk, 128 co]
            wdr = {}
            for co in range(NCH):
                for kh in range(KS):
                    for kw in range(KS):
                        wdr[(kh, kw, co)] = constp.tile(
                            [128, NCH * 128], FP8,
                            tag=f"wdr{co}_{kh}_{kw}",
                            name=f"wdr{co}_{kh}_{kw}",
                        )
            with tc.tile_pool(name="pswt", bufs=2, space="PSUM") as pswt:
                for co in range(NCH):
                    wraw = wsbp.tile([128, C * KS * KS], F32, tag=f"wraw{co}")
                    nc.sync.dma_start(wraw[:], wv[co * 128:(co + 1) * 128, :])
                    wbin = wsbp.tile([128, C * KS * KS], BF16, tag=f"wbin{co}")
                    nc.scalar.activation(wbin[:], wraw[:], SIGN)
                    wb3 = wbin[:].rearrange("p (c k) -> p c k", c=C)
                    for ci in range(NCH):
                        for kh in range(KS):
                            for kw in range(KS):
                                pt = pswt.tile([128, 128], BF16, tag="tp")
                                src = wb3[:, ci * 128:(ci + 1) * 128, kh * KS + kw]
                                nc.tensor.transpose(pt[:], src, ident[:])
                                nc.vector.tensor_copy(
                                    wdr[(kh, kw, co)][:, ci * 128:(ci + 1) * 128],
                                    pt[:],
                                )

            # Padded binarized activations: one fp8 tile per image holding
            # both ci chunks: [128, 2 * 58*64], interior rows 1..57 cols 1..57
            apad = {}
            for img in range(BL):
                ap_t = apadp.tile([128, NCH * PADF], FP8, tag=f"ap{img}")
                nc.gpsimd.memset(ap_t[:], 0.0)
                a4 = ap_t[:].rearrange("p (c h w) -> p c h w", c=NCH, h=PH)
                for ci in range(NCH):
                    xs_t = xsp.tile([128, NPIX], F32, tag="xs")
                    nc.sync.dma_start(xs_t[:], xv[img, ci * 128:(ci + 1) * 128, :])
                    nc.scalar.activation(
                        a4[:, ci, 1:H + 1, 1:W + 1],
                        xs_t[:].rearrange("p (h w) -> p h w", h=H),
                        SIGN,
                        bias=neg_alpha[:, ci:ci + 1],
                    )
                apad[img] = ap_t

            # Conv: 9 DoubleRow matmuls (one per tap) accumulated in PSUM
            n_acc = KS * KS
            with tc.tile_pool(name="psc", bufs=8, space="PSUM") as psc:
                for img in range(BL):
                    a4 = apad[img][:].rearrange("p (c h w) -> p c h w", c=NCH, h=PH)
                    for co in range(NCH):
                        pts = [
                            psc.tile([128, NFREE], F32, tag="conv", name=f"conv{sp}")
                            for sp in range(NSP)
                        ]
                        i_acc = 0
                        for kh in range(KS):
                            for kw in range(KS):
                                w3 = wdr[(kh, kw, co)][:].rearrange(
                                    "p (c m) -> p c m", c=NCH
                                )
                                first = i_acc == 0
                                last = i_acc == n_acc - 1
                                for sp in range(NSP):
                                    r0 = sp * RPC + kh
                                    rhs = a4[:, :, r0:r0 + RPC, kw:kw + W]
                                    nc.tensor.matmul(
                                        pts[sp][:], w3, rhs,
                                        start=first, stop=last, perf_mode=DR,
                                    )
                                i_acc += 1
                        ot = outsp.tile([128, NPIX], F32, tag="out")
                        for sp in range(NSP):
                            nc.vector.tensor_copy(
                                ot[:, sp * NFREE:(sp + 1) * NFREE], pts[sp][:]
                            )
                        nc.sync.dma_start(
                            ov[img, co * 128:(co + 1) * 128, :], ot[:]
                        )
    _split_excess_waits(nc)
    return nc


_prog_cache = {}


def _get_program() -> bass.Bass:
    if "nc" not in _prog_cache:
        _prog_cache["nc"] = _build_program()
    return _prog_cache["nc"]


def _run(x, alpha, weight, trace=False):
    x = np.ascontiguousarray(np.asarray(x, dtype=np.float32))
    alpha = np.ascontiguousarray(np.asarray(alpha, dtype=np.float32).reshape(C))
    weight = np.ascontiguousarray(np.asarray(weight, dtype=np.float32))
    assert x.shape == (B, C, H, W) and weight.shape == (C, C, KS, KS)

    nc = _get_program()
    in_maps = [
        {
            "x": np.ascontiguousarray(x[i * BL:(i + 1) * BL]),
            "alpha": alpha,
            "weight": weight,
        }
        for i in range(N_CORES)
    ]
    res = run_bass_kernel_spmd(nc, in_maps, list(range(N_CORES)), trace=trace)
    out = np.concatenate([res.results[i]["out"] for i in range(N_CORES)], axis=0)
    return out.astype(np.float32, copy=False), res


def kernel(x, alpha, weight):
    out, _ = _run(x, alpha, weight, trace=False)
    return out


def kernel_timed(x, alpha, weight):
    out, res = _run(x, alpha, weight, trace=True)
    return out, res


# revision 10
# speedup vs baseline: 1.6439x; 1.0000x over previous
"""XNOR/ReActNet binarized 3x3 conv on 8 Trainium2 NeuronCores.

out = conv2d(sign(x - alpha), sign(weight), stride 1, pad 1)
  x      [32, 256, 56, 56] f32
  alpha  [256, 1, 1]       f32
  weight [256, 256, 3, 3]  f32
  out    [32, 256, 56, 56] f32

Strategy (data-parallel): each core takes 4 images. Binarized values are
exactly +-1, so they are exact in fp8e4; the conv runs as 9 shifted
matmuls (one per kernel tap) in fp8 DoubleRow mode (contraction over all
256 input channels per matmul: 128 partitions x 2 k-tiles), accumulating
in fp32 PSUM. All sums are small integers -> bit-exact vs the reference.

Per core:
  - weights: DMA [128co, 2304] f32, Sign->bf16, PE-transpose to [ci,co]
    128x128 tiles, pack as fp8 [128, 2(ci chunk), 128co] per (kh,kw,co).
  - activations: DMA [128ci, 3136] f32 per (img, ci_chunk); Sign(x-alpha)
    written fp8 into the interior of a zeroed row-padded image
    [58 rows x 64 cols] (row stride 64 keeps the DoubleRow k-tile step
    16B-aligned); both ci chunks live in one tile -> 4D windowed rhs AP.
  - conv: per (img, co_chunk): 7 PSUM tiles [128, 8*56]; 9 accumulating
    DoubleRow matmuls each; DVE copy PSUM->SBUF; one 1.6MB DMA out.
"""

import numpy as np

import concourse.bass as bass
import concourse.mybir as mybir
import concourse.tile as tile
from concourse.masks import make_identity
from concourse.bass_utils import run_bass_kernel_spmd

N_CORES = 8
B, C, H, W, KS = 32, 256, 56, 56, 3
BL = B // N_CORES           # images per core
PH, PW = H + 2, 64          # padded rows, row stride (58 x 64)
PADF = PH * PW              # 3712 (16B aligned for DoubleRow k-tile step)
NPIX = H * W                # 3136
RPC = 8                     # output rows per PSUM tile
NSP = H // RPC              # 7 spatial chunks
NFREE = RPC * W             # 448 (fits one 2KB f32 PSUM bank)
NCH = C // 128              # 2 channel chunks
F32 = mybir.dt.float32
BF16 = mybir.dt.bfloat16
FP8 = mybir.dt.float8e4
SIGN = mybir.ActivationFunctionType.Sign
DR = mybir.MatmulPerfMode.DoubleRow


def _split_excess_waits(nc):
    """This walrus build rejects instructions carrying more than one sem
    wait ("Too many sync wait commands" from setupSyncWait). Tile's
    scheduler can attach several. Hoist the excess onto same-engine NoOps
    placed just before the instruction — engines are in-order, so the
    semantics are identical."""
    k = 0
    for f in nc.m.functions:
        for bb in f.blocks:
            old = list(bb.instructions)
            new = []
            changed = False
            for ins in old:
                si = ins.sync_info
                waits = list(si.on_wait) if si and si.on_wait else []
                if len(waits) > 1:
                    for w in waits[:-1]:
                        nop = mybir.InstNoOp(
                            name=f"I-wsplit{k}",
                            ins=[],
                            outs=[],
                            engine=ins.engine,
                            sync_info=mybir.SyncInfo(on_wait=[w], on_update=[]),
                        )
                        k += 1
                        new.append(nop)
                    si.on_wait = waits[-1:]
                    changed = True
                new.append(ins)
            if changed:
                bb.instructions[:] = new


def _build_program() -> bass.Bass:
    nc = bass.Bass()
    x = nc.dram_tensor("x", [BL, C, H, W], F32, kind="ExternalInput")
    alpha = nc.dram_tensor("alpha", [C], F32, kind="ExternalInput")
    weight = nc.dram_tensor("weight", [C, C, KS, KS], F32, kind="ExternalInput")
    out = nc.dram_tensor("out", [BL, C, H, W], F32, kind="ExternalOutput")

    xv = x[:].rearrange("b c h w -> b c (h w)")
    wv = weight[:].rearrange("o i kh kw -> o (i kh kw)")
    ov = out[:].rearrange("b c h w -> b c (h w)")

    with tile.TileContext(nc) as tc:
        with (
            tc.tile_pool(name="const", bufs=1) as constp,
            tc.tile_pool(name="apad", bufs=1) as apadp,
            tc.tile_pool(name="wsb", bufs=1) as wsbp,
            tc.tile_pool(name="xs", bufs=3) as xsp,
            tc.tile_pool(name="outs", bufs=2) as outsp,
        ):
            ident = constp.tile([128, 128], BF16, tag="ident")
            make_identity(nc, ident[:])

            alpha_sb = constp.tile([128, NCH], F32, tag="alpha")
            nc.sync.dma_start(alpha_sb[:], alpha[:].rearrange("(n p) -> p n", p=128))
            neg_alpha = constp.tile([128, NCH], F32, tag="nalpha")
            nc.scalar.mul(neg_alpha[:], alpha_sb[:], -1.0)

            # Binarized transposed weights packed for DoubleRow:
            # wdr[(kh,kw,co)] = fp8 [128 ci_local, 2 ci_chunk, 128 co]
            wdr = {}
            for co in range(NCH):
                for kh in range(KS):
                    for kw in range(KS):
                        wdr[(kh, kw, co)] = constp.tile(
                            [128, NCH * 128], FP8,
                            tag=f"wdr{co}_{kh}_{kw}",
                            name=f"wdr{co}_{kh}_{kw}",
                        )

            def prep_weights(co, pswt):
                wraw = wsbp.tile([128, C * KS * KS], F32, tag=f"wraw{co}",
                                 name=f"wraw{co}")
                nc.sync.dma_start(wraw[:], wv[co * 128:(co + 1) * 128, :])
                wbin = wsbp.tile([128, C * KS * KS], BF16, tag=f"wbin{co}",
                                 name=f"wbin{co}")
                nc.scalar.activation(wbin[:], wraw[:], SIGN)
                wb3 = wbin[:].rearrange("p (c k) -> p c k", c=C)
                for ci in range(NCH):
                    for kh in range(KS):
                        for kw in range(KS):
                            pt = pswt.tile([128, 128], BF16, tag="tp")
                            src = wb3[:, ci * 128:(ci + 1) * 128, kh * KS + kw]
                            nc.tensor.transpose(pt[:], src, ident[:])
                            nc.vector.tensor_copy(
                                wdr[(kh, kw, co)][:, ci * 128:(ci + 1) * 128],
                                pt[:],
                            )

            # Padded binarized activations: one fp8 tile per image holding
            # both ci chunks: [128, 2 * 58*64], interior rows 1..57 cols 1..57.
            # Each chunk's DMA is split in half so binarize starts earlier.
            apad = {}
            HH = H // 2

            def prep_image(img):
                ap_t = apadp.tile([128, NCH * PADF], FP8, tag=f"ap{img}",
                                  name=f"ap{img}")
                nc.gpsimd.memset(ap_t[:], 0.0)
                a4 = ap_t[:].rearrange("p (c h w) -> p c h w", c=NCH, h=PH)
                x3 = xv[img].rearrange("c (h w) -> c h w", h=H)
                for ci in range(NCH):
                    for half in range(2):
                        rs = half * HH
                        xs_t = xsp.tile([128, HH * W], F32, tag="xs")
                        nc.sync.dma_start(
                            xs_t[:],
                            x3[ci * 128:(ci + 1) * 128, rs:rs + HH, :],
                        )
                        nc.scalar.activation(
                            a4[:, ci, 1 + rs:1 + rs + HH, 1:W + 1],
                            xs_t[:].rearrange("p (h w) -> p h w", h=HH),
                            SIGN,
                            bias=neg_alpha[:, ci:ci + 1],
                        )
                apad[img] = ap_t

            # Emission order: co0 weights (warm PE, unblock first group),
            # img0, then remaining weights/images interleaved.
            with tc.tile_pool(name="pswt", bufs=2, space="PSUM") as pswt:
                prep_weights(0, pswt)
                prep_image(0)
                prep_weights(1, pswt)
            for img in range(1, BL):
                prep_image(img)

            # Conv: 9 DoubleRow matmuls (one per tap) accumulated in PSUM
            n_acc = KS * KS
            with tc.tile_pool(name="psc", bufs=8, space="PSUM") as psc:
                for img in range(BL):
                    a4 = apad[img][:].rearrange("p (c h w) -> p c h w", c=NCH, h=PH)
                    for co in range(NCH):
                        pts = [
                            psc.tile([128, NFREE], F32, tag="conv", name=f"conv{sp}")
                            for sp in range(NSP)
                        ]
                        i_acc = 0
                        for kh in range(KS):
                            for kw in range(KS):
                                w3 = wdr[(kh, kw, co)][:].rearrange(
                                    "p (c m) -> p c m", c=NCH
                                )
                                first = i_acc == 0
                                last = i_acc == n_acc - 1
                                for sp in range(NSP):
                                    r0 = sp * RPC + kh
                                    rhs = a4[:, :, r0:r0 + RPC, kw:kw + W]
                                    nc.tensor.matmul(
                                        pts[sp][:], w3, rhs,
                                        start=first, stop=last, perf_mode=DR,
                                    )
                                i_acc += 1
                        ot = outsp.tile([128, NPIX], F32, tag="out")
                        for sp in range(NSP):
                            nc.vector.tensor_copy(
                                ot[:, sp * NFREE:(sp + 1) * NFREE], pts[sp][:]
                            )
                            nc.sync.dma_start(
                                ov[img, co * 128:(co + 1) * 128,
                                   sp * NFREE:(sp + 1) * NFREE],
                                ot[:, sp * NFREE:(sp + 1) * NFREE],
                            )
    _split_excess_waits(nc)
    return nc


_prog_cache = {}


def _get_program() -> bass.Bass:
    if "nc" not in _prog_cache:
        _prog_cache["nc"] = _build_program()
    return _prog_cache["nc"]


def _run(x, alpha, weight, trace=False):
    x = np.ascontiguousarray(np.asarray(x, dtype=np.float32))
    alpha = np.ascontiguousarray(np.asarray(alpha, dtype=np.float32).reshape(C))
    weight = np.ascontiguousarray(np.asarray(weight, dtype=np.float32))
    assert x.shape == (B, C, H, W) and weight.shape == (C, C, KS, KS)

    nc = _get_program()
    in_maps = [
        {
            "x": np.ascontiguousarray(x[i * BL:(i + 1) * BL]),
            "alpha": alpha,
            "weight": weight,
        }
        for i in range(N_CORES)
    ]
    res = run_bass_kernel_spmd(nc, in_maps, list(range(N_CORES)), trace=trace)
    out = np.concatenate([res.results[i]["out"] for i in range(N_CORES)], axis=0)
    return out.astype(np.float32, copy=False), res


def kernel(x, alpha, weight):
    out, _ = _run(x, alpha, weight, trace=False)
    return out


def kernel_timed(x, alpha, weight):
    out, res = _run(x, alpha, weight, trace=True)
    return out, res


# revision 17
# speedup vs baseline: 1.7144x; 1.0429x over previous
"""XNOR/ReActNet binarized 3x3 conv on 8 Trainium2 NeuronCores.

out = conv2d(sign(x - alpha), sign(weight), stride 1, pad 1)
  x      [32, 256, 56, 56] f32
  alpha  [256, 1, 1]       f32
  weight [256, 256, 3, 3]  f32
  out    [32, 256, 56, 56] f32

Strategy (data-parallel): each core takes 4 images. Binarized values are
exactly +-1, so they are exact in fp8e4; the conv runs as 9 shifted
matmuls (one per kernel tap) in fp8 DoubleRow mode (contraction over all
256 input channels per matmul: 128 partitions x 2 k-tiles), accumulating
in fp32 PSUM. All sums are small integers -> bit-exact vs the reference.

Per core:
  - weights: DMA [128co, 2304] f32, Sign->bf16, PE-transpose to [ci,co]
    128x128 tiles, pack as fp8 [128, 2(ci chunk), 128co] per (kh,kw,co).
  - activations: DMA [128ci, 3136] f32 per (img, ci_chunk); Sign(x-alpha)
    written fp8 into the interior of a zeroed row-padded image
    [58 rows x 64 cols] (row stride 64 keeps the DoubleRow k-tile step
    16B-aligned); both ci chunks live in one tile -> 4D windowed rhs AP.
  - conv: per (img, co_chunk): 7 PSUM tiles [128, 8*56]; 9 accumulating
    DoubleRow matmuls each; DVE copy PSUM->SBUF; one 1.6MB DMA out.
"""

import numpy as np

import concourse.bass as bass
import concourse.mybir as mybir
import concourse.tile as tile
from concourse.masks import make_identity
from concourse.bass_utils import run_bass_kernel_spmd

N_CORES = 8
B, C, H, W, KS = 32, 256, 56, 56, 3
BL = B // N_CORES           # images per core
PH, PW = H + 2, 64          # padded rows, row stride (58 x 64)
PADF = PH * PW              # 3712 (16B aligned for DoubleRow k-tile step)
NPIX = H * W                # 3136
RPC = 8                     # output rows per PSUM tile
NSP = H // RPC              # 7 spatial chunks
NFREE = RPC * W             # 448 (fits one 2KB f32 PSUM bank)
NCH = C // 128              # 2 channel chunks
F32 = mybir.dt.float32
BF16 = mybir.dt.bfloat16
FP8 = mybir.dt.float8e4
SIGN = mybir.ActivationFunctionType.Sign
DR = mybir.MatmulPerfMode.DoubleRow


def _split_excess_waits(nc):
    """This walrus build rejects instructions carrying more than one sem
    wait ("Too many sync wait commands" from setupSyncWait). Tile's
    scheduler can attach several. Hoist the excess onto same-engine NoOps
    placed just before the instruction — engines are in-order, so the
    semantics are identical."""
    k = 0
    for f in nc.m.functions:
        for bb in f.blocks:
            old = list(bb.instructions)
            new = []
            changed = False
            for ins in old:
                si = ins.sync_info
                waits = list(si.on_wait) if si and si.on_wait else []
                if len(waits) > 1:
                    for w in waits[:-1]:
                        nop = mybir.InstNoOp(
                            name=f"I-wsplit{k}",
                            ins=[],
                            outs=[],
                            engine=ins.engine,
                            sync_info=mybir.SyncInfo(on_wait=[w], on_update=[]),
                        )
                        k += 1
                        new.append(nop)
                    si.on_wait = waits[-1:]
                    changed = True
                new.append(ins)
            if changed:
                bb.instructions[:] = new


import os as _os
_SP_OUTER = _os.environ.get("K_SP_OUTER", "1") == "1"
_BORDER_MEMSET = _os.environ.get("K_BORDER_MEMSET", "1") == "1"
_SPLIT_WSIGN = _os.environ.get("K_SPLIT_WSIGN", "1") == "1"


def _build_program() -> bass.Bass:
    nc = bass.Bass()
    x = nc.dram_tensor("x", [BL, C, H, W], F32, kind="ExternalInput")
    alpha = nc.dram_tensor("alpha", [C], F32, kind="ExternalInput")
    weight = nc.dram_tensor("weight", [C, C, KS, KS], F32, kind="ExternalInput")
    out = nc.dram_tensor("out", [BL, C, H, W], F32, kind="ExternalOutput")

    xv = x[:].rearrange("b c h w -> b c (h w)")
    wv = weight[:].rearrange("o i kh kw -> o (i kh kw)")
    ov = out[:].rearrange("b c h w -> b c (h w)")

    with tile.TileContext(nc) as tc:
        with (
            tc.tile_pool(name="const", bufs=1) as constp,
            tc.tile_pool(name="apad", bufs=1) as apadp,
            tc.tile_pool(name="wsb", bufs=1) as wsbp,
            tc.tile_pool(name="xs", bufs=3) as xsp,
            tc.tile_pool(name="outs", bufs=2) as outsp,
        ):
            ident = constp.tile([128, 128], BF16, tag="ident")
            make_identity(nc, ident[:])

            alpha_sb = constp.tile([128, NCH], F32, tag="alpha")
            nc.sync.dma_start(alpha_sb[:], alpha[:].rearrange("(n p) -> p n", p=128))
            neg_alpha = constp.tile([128, NCH], F32, tag="nalpha")
            nc.scalar.mul(neg_alpha[:], alpha_sb[:], -1.0)

            # Binarized transposed weights packed for DoubleRow:
            # wdr[(kh,kw,co)] = fp8 [128 ci_local, 2 ci_chunk, 128 co]
            wdr = {}
            for co in range(NCH):
                for kh in range(KS):
                    for kw in range(KS):
                        wdr[(kh, kw, co)] = constp.tile(
                            [128, NCH * 128], FP8,
                            tag=f"wdr{co}_{kh}_{kw}",
                            name=f"wdr{co}_{kh}_{kw}",
                        )

            HKK = (C // NCH) * KS * KS  # columns per ci chunk in wraw

            def prep_weights(co, pswt):
                wraw = wsbp.tile([128, C * KS * KS], F32, tag=f"wraw{co}",
                                 name=f"wraw{co}")
                nc.sync.dma_start(wraw[:], wv[co * 128:(co + 1) * 128, :])
                wbin = wsbp.tile([128, C * KS * KS], BF16, tag=f"wbin{co}",
                                 name=f"wbin{co}")
                wb3 = wbin[:].rearrange("p (c k) -> p c k", c=C)
                if not _SPLIT_WSIGN:
                    nc.scalar.activation(wbin[:], wraw[:], SIGN)
                for ci in range(NCH):
                    if _SPLIT_WSIGN:
                        nc.scalar.activation(
                            wbin[:, ci * HKK:(ci + 1) * HKK],
                            wraw[:, ci * HKK:(ci + 1) * HKK],
                            SIGN,
                        )
                    for kh in range(KS):
                        for kw in range(KS):
                            pt = pswt.tile([128, 128], BF16, tag="tp")
                            src = wb3[:, ci * 128:(ci + 1) * 128, kh * KS + kw]
                            nc.tensor.transpose(pt[:], src, ident[:])
                            nc.vector.tensor_copy(
                                wdr[(kh, kw, co)][:, ci * 128:(ci + 1) * 128],
                                pt[:],
                            )

            # Padded binarized activations: one fp8 tile per image holding
            # both ci chunks: [128, 2 * 58*64], interior rows 1..57 cols 1..57.
            # Each chunk's DMA is split in half so binarize starts earlier.
            apad = {}
            HH = H // 2

            def prep_image(img):
                ap_t = apadp.tile([128, NCH * PADF], FP8, tag=f"ap{img}",
                                  name=f"ap{img}")
                a4 = ap_t[:].rearrange("p (c h w) -> p c h w", c=NCH, h=PH)
                x3 = xv[img].rearrange("c (h w) -> c h w", h=H)
                if not _BORDER_MEMSET:
                    nc.gpsimd.memset(ap_t[:], 0.0)
                for ci in range(NCH):
                    if _BORDER_MEMSET:
                        # only the 1-pixel border is ever read as zero-pad;
                        # cols 58..63 of the 64-col row stride never read
                        nc.gpsimd.memset(a4[:, ci, 0, :], 0.0)
                        nc.gpsimd.memset(a4[:, ci, PH - 1, :], 0.0)
                        nc.gpsimd.memset(a4[:, ci, :, 0], 0.0)
                        nc.gpsimd.memset(a4[:, ci, :, W + 1], 0.0)
                    for half in range(2):
                        rs = half * HH
                        xs_t = xsp.tile([128, HH * W], F32, tag="xs")
                        nc.sync.dma_start(
                            xs_t[:],
                            x3[ci * 128:(ci + 1) * 128, rs:rs + HH, :],
                        )
                        nc.scalar.activation(
                            a4[:, ci, 1 + rs:1 + rs + HH, 1:W + 1],
                            xs_t[:].rearrange("p (h w) -> p h w", h=HH),
                            SIGN,
                            bias=neg_alpha[:, ci:ci + 1],
                        )
                apad[img] = ap_t

            # Emission order: co0 weights (warm PE, unblock first group),
            # img0, then remaining weights/images interleaved.
            with tc.tile_pool(name="pswt", bufs=2, space="PSUM") as pswt:
                prep_weights(0, pswt)
                prep_image(0)
                prep_weights(1, pswt)
            for img in range(1, BL):
                prep_image(img)

            # Conv: per spatial chunk, 9 DoubleRow matmuls (one per tap)
            # accumulated in PSUM. sp-outer order lets the first matmuls
            # start once the top half-image is binarized, and drains each
            # PSUM tile (copy + DMA) right after its 9th accumulation.
            n_acc = KS * KS
            with tc.tile_pool(name="psc", bufs=8, space="PSUM") as psc:
                for img in range(BL):
                    a4 = apad[img][:].rearrange("p (c h w) -> p c h w", c=NCH, h=PH)
                    for co in range(NCH):
                        ot = outsp.tile([128, NPIX], F32, tag="out")
                        if _SP_OUTER:
                            for sp in range(NSP):
                                pt = psc.tile([128, NFREE], F32, tag="conv")
                                i_acc = 0
                                for kh in range(KS):
                                    for kw in range(KS):
                                        w3 = wdr[(kh, kw, co)][:].rearrange(
                                            "p (c m) -> p c m", c=NCH
                                        )
                                        r0 = sp * RPC + kh
                                        rhs = a4[:, :, r0:r0 + RPC, kw:kw + W]
                                        nc.tensor.matmul(
                                            pt[:], w3, rhs,
                                            start=i_acc == 0,
                                            stop=i_acc == n_acc - 1,
                                            perf_mode=DR,
                                        )
                                        i_acc += 1
                                nc.vector.tensor_copy(
                                    ot[:, sp * NFREE:(sp + 1) * NFREE], pt[:]
                                )
                                nc.sync.dma_start(
                                    ov[img, co * 128:(co + 1) * 128,
                                       sp * NFREE:(sp + 1) * NFREE],
                                    ot[:, sp * NFREE:(sp + 1) * NFREE],
                                )
                        else:
                            pts = [
                                psc.tile([128, NFREE], F32, tag="conv",
                                         name=f"conv{sp}")
                                for sp in range(NSP)
                            ]
                            i_acc = 0
                            for kh in range(KS):
                                for kw in range(KS):
                                    w3 = wdr[(kh, kw, co)][:].rearrange(
                                        "p (c m) -> p c m", c=NCH
                                    )
                                    first = i_acc == 0
                                    last = i_acc == n_acc - 1
                                    for sp in range(NSP):
                                        r0 = sp * RPC + kh
                                        rhs = a4[:, :, r0:r0 + RPC, kw:kw + W]
                                        nc.tensor.matmul(
                                            pts[sp][:], w3, rhs,
                                            start=first, stop=last,
                                            perf_mode=DR,
                                        )
                                    i_acc += 1
                            for sp in range(NSP):
                                nc.vector.tensor_copy(
                                    ot[:, sp * NFREE:(sp + 1) * NFREE],
                                    pts[sp][:],
                                )
                                nc.sync.dma_start(
                                    ov[img, co * 128:(co + 1) * 128,
                                       sp * NFREE:(sp + 1) * NFREE],
                                    ot[:, sp * NFREE:(sp + 1) * NFREE],
                                )
    _split_excess_waits(nc)
    return nc


_prog_cache = {}


def _get_program() -> bass.Bass:
    if "nc" not in _prog_cache:
        _prog_cache["nc"] = _build_program()
    return _prog_cache["nc"]


def _run(x, alpha, weight, trace=False):
    x = np.ascontiguousarray(np.asarray(x, dtype=np.float32))
    alpha = np.ascontiguousarray(np.asarray(alpha, dtype=np.float32).reshape(C))
    weight = np.ascontiguousarray(np.asarray(weight, dtype=np.float32))
    assert x.shape == (B, C, H, W) and weight.shape == (C, C, KS, KS)

    nc = _get_program()
    in_maps = [
        {
            "x": np.ascontiguousarray(x[i * BL:(i + 1) * BL]),
            "alpha": alpha,
            "weight": weight,
        }
        for i in range(N_CORES)
    ]
    res = run_bass_kernel_spmd(nc, in_maps, list(range(N_CORES)), trace=trace)
    out = np.concatenate([res.results[i]["out"] for i in range(N_CORES)], axis=0)
    return out.astype(np.float32, copy=False), res


def kernel(x, alpha, weight):
    out, _ = _run(x, alpha, weight, trace=False)
    return out


def kernel_timed(x, alpha, weight):
    out, res = _run(x, alpha, weight, trace=True)
    return out, res
